# revision 10
# baseline (speedup 1.0000x reference)
"""NeRF coarse+fine forward pass on 8 Trainium2 NeuronCores.

Pure data-parallel SPMD: 256 rays per core, one Bass program, no collectives.

Layouts:
  - per-ray ops (z strata, compositing, CDF sampling, bitonic sort):
      [128 rays (partitions) x samples (free)], 2 ray-chunks per core
  - MLP: channel-major [channels (partitions) x samples (free)];
      coarse tiles of 512 samples (8 rays), fine tiles of 384 samples (2 rays)
  - positional encoding computed by the PE (scale matrix K=3), Cody-Waite
      range reduction on DVE/GPSIMD, ACT Sin; cos(x) emitted as -cos via
      sin(|r|-pi/2) with the sign folded into host-negated weight rows.
"""
import numpy as np

import concourse.bass as bass
import concourse.mybir as mybir
import concourse.tile as tile
from concourse import bacc

F32 = mybir.dt.float32
AF = mybir.ActivationFunctionType
ALU = mybir.AluOpType
AX = mybir.AxisListType

# problem constants (hardcoded per harness contract)
N_RAYS_FULL = 2048
N_CORES = 8
N_RAYS = N_RAYS_FULL // N_CORES          # 256 rays per core
N_CHUNK = 2                              # ray chunks of 128
NC_ = 64                                 # coarse samples
NF_ = 128                                # fine new samples
NT_ = NC_ + NF_                          # 192 fine-pass samples
NSORT = 256                              # bitonic width
L_XYZ, L_DIR = 10, 4
W = 256
NEAR, FAR = 2.0, 6.0
CTILE = 512                              # coarse sample tile (8 rays)
FTILE = 384                              # fine sample tile (2 rays)

TWO_PI = 2.0 * np.pi
INV_2PI = float(np.float32(1.0 / TWO_PI))
MAGIC = float(np.float32(1.5 * 2 ** 23))
CW1 = float(np.float32(6.283203125))     # 2pi to 12 bits; k*CW1 exact for k < 2^12
CW2 = float(np.float32(TWO_PI - 6.283203125))
PI = float(np.float32(np.pi))
PIH = float(np.float32(np.pi / 2))
BIG = 1e10


# ----------------------------------------------------------------- host prep
def _perm_pe():
    """Map my pe row -> (ref channel, sign). My rows: 0-29 sin(l,c), 30-31 pad,
    32-61 -cos(l,c), 62-63 pad, 64-66 xyz."""
    rows = []
    for l in range(L_XYZ):
        for c in range(3):
            rows.append((3 + 6 * l + c, 1.0))        # sin
    rows.append(None); rows.append(None)
    for l in range(L_XYZ):
        for c in range(3):
            rows.append((3 + 6 * l + 3 + c, -1.0))   # value is -cos
    rows.append(None); rows.append(None)
    for c in range(3):
        rows.append((c, 1.0))
    return rows                                       # 67 entries


def _perm_ve():
    """ve rows: 0-11 sin(l,c), 12-31 pad, 32-43 -cos(l,c), 44-63 pad, 64-66 xyz."""
    rows = []
    for l in range(L_DIR):
        for c in range(3):
            rows.append((3 + 6 * l + c, 1.0))
    rows += [None] * (32 - len(rows))
    for l in range(L_DIR):
        for c in range(3):
            rows.append((3 + 6 * l + 3 + c, -1.0))
    rows += [None] * (64 - len(rows))
    for c in range(3):
        rows.append((c, 1.0))
    return rows                                       # 67 entries


def _expand_rows(w_ref, perm):
    """w_ref [C_ref, M] -> w' [len(perm), M] with my row order/signs."""
    out = np.zeros((len(perm), w_ref.shape[1]), np.float32)
    for i, ent in enumerate(perm):
        if ent is not None:
            ch, sgn = ent
            out[i] = sgn * w_ref[ch]
    return out


def _enc_lhsT(levels, rows):
    """lhsT [3, rows]: trig args 2^l * coord for (l,c) packed rows, zero-padded."""
    m = np.zeros((rows, 3), np.float32)
    for l in range(levels):
        for c in range(3):
            m[3 * l + c, c] = np.float32(2.0 ** l)
    return m.T.copy()


def _host_weights(params):
    """Flatten one net's params into the DRAM tensors the kernel expects."""
    pe_perm, ve_perm = _perm_pe(), _perm_ve()
    t = {}
    full = params["full"]
    w0, b0 = [np.asarray(a, np.float32) for a in full[0]]
    t["w0"] = _expand_rows(w0, pe_perm)               # [67, 256]
    t["b0"] = b0.reshape(-1, 1)
    for j in range(1, 8):
        wj, bj = [np.asarray(a, np.float32) for a in full[j]]
        if j == 5:  # skip layer: input [pe(63); o(256)]
            t["w5pe"] = _expand_rows(wj[:63], pe_perm)     # [67, 256]
            t["w5o"] = wj[63:].copy()                      # [256, 256]
        else:
            t[f"w{j}"] = wj.copy()
        t[f"b{j}"] = bj.reshape(-1, 1)
    wa, ba = [np.asarray(a, np.float32) for a in params["alpha"]]
    t["wa"], t["ba"] = wa.copy(), float(ba[0])
    wf, bf = [np.asarray(a, np.float32) for a in params["feature"]]
    t["wf"], t["bf"] = wf.copy(), bf.reshape(-1, 1)
    wv, bv = [np.asarray(a, np.float32) for a in params["view"]]
    t["wvf"] = wv[:256].copy()                        # feat part [256, 128]
    t["wvv"] = _expand_rows(wv[256:], ve_perm)        # [67, 128]
    t["bv"] = bv.reshape(-1, 1)
    wr, br = [np.asarray(a, np.float32) for a in params["rgb"]]
    t["wr"], t["br"] = wr.copy(), br.reshape(-1, 1)
    return t


def _strata_rows():
    t = np.linspace(0.0, 1.0, NC_, dtype=np.float32)
    z = (NEAR * (1.0 - t) + FAR * t).astype(np.float32)
    mids = (0.5 * (z[:-1] + z[1:])).astype(np.float32)
    lower = np.concatenate([z[:1], mids]).astype(np.float32)
    upper = np.concatenate([mids, z[-1:]]).astype(np.float32)
    width = (upper - lower).astype(np.float32)
    return lower, width


# --------------------------------------------------------------- bass program
def build_program(debug=False):
    nc = bacc.Bacc("TRN2", target_bir_lowering=False, debug=debug)

    def din(name, shape):
        return nc.dram_tensor(name, shape, F32, kind="ExternalInput").ap()

    ins = {
        "rays_o": din("rays_o", [N_RAYS, 3]),
        "rays_d": din("rays_d", [N_RAYS, 3]),
        "viewdirs": din("viewdirs", [N_RAYS, 3]),
        "dnorm": din("dnorm", [N_RAYS, 1]),
        "t_rand": din("t_rand", [N_RAYS, NC_]),
        "u_samples": din("u_samples", [N_RAYS, NF_]),
        "lower_rep": din("lower_rep", [128, NC_]),
        "width_rep": din("width_rep", [128, NC_]),
        "enc_pe": din("enc_pe", [3, 32]),
        "enc_ve": din("enc_ve", [3, 32]),
    }
    wnames = (["w0", "b0"] + [f"w{j}" for j in range(1, 8) if j != 5] +
              ["w5pe", "w5o"] + [f"b{j}" for j in range(1, 8)] +
              ["wa", "wf", "bf", "wvf", "wvv", "bv", "wr", "br"])
    wshapes = {"w0": [67, W], "b0": [W, 1], "w5pe": [67, W], "w5o": [W, W],
               "wa": [W, 1], "wf": [W, W], "bf": [W, 1],
               "wvf": [W, 128], "wvv": [67, 128], "bv": [128, 1],
               "wr": [128, 3], "br": [3, 1]}
    for j in range(1, 8):
        if j != 5:
            wshapes[f"w{j}"] = [W, W]
        wshapes[f"b{j}"] = [W, 1]
    for net in ("c", "f"):
        for wn in wnames:
            ins[f"{net}_{wn}"] = din(f"{net}_{wn}", wshapes[wn])
    ba_c = None  # alpha biases passed as floats at build time? no — via dram [1,1]
    ins["c_ba_s"] = din("c_ba_s", [1, 1])
    ins["f_ba_s"] = din("f_ba_s", [1, 1])

    out = nc.dram_tensor("out", [N_RAYS, 8], F32, kind="ExternalOutput").ap()

    with tile.TileContext(nc) as tc:
        _build_kernel(nc, tc, ins, out)

    nc.compile()
    return nc


def _build_kernel(nc, tc, ins, out):
    from contextlib import ExitStack
    ctx = ExitStack()
    with ctx:
        wpool = ctx.enter_context(tc.tile_pool(name="w", bufs=1))
        cpool = ctx.enter_context(tc.tile_pool(name="chunk", bufs=1))
        epool = ctx.enter_context(tc.tile_pool(name="enc", bufs=2))
        apool = ctx.enter_context(tc.tile_pool(name="act", bufs=2))
        spool = ctx.enter_context(tc.tile_pool(name="stage", bufs=3))
        mpool = ctx.enter_context(tc.tile_pool(name="mask", bufs=2))
        sopool = ctx.enter_context(tc.tile_pool(name="sort", bufs=2))
        pp = ctx.enter_context(tc.tile_pool(name="ps", bufs=4, space="PSUM"))
        pp_enc = ctx.enter_context(tc.tile_pool(name="pse", bufs=2, space="PSUM"))
        pp_sm = ctx.enter_context(tc.tile_pool(name="pssm", bufs=1, space="PSUM"))

        # ---------------- load weights & constants (split >128-row tensors)
        wt = {}
        for key, ap in ins.items():
            if key in ("rays_o", "rays_d", "viewdirs", "dnorm", "t_rand", "u_samples"):
                continue
            shape = list(ap.shape)
            if shape[0] > 128:
                chunks = []
                for k0 in range(0, shape[0], 128):
                    ksz = min(128, shape[0] - k0)
                    t = wpool.tile([ksz, shape[1]], F32, tag=f"wt_{key}_{k0}")
                    nc.sync.dma_start(t[:], ap[k0:k0 + ksz, :])
                    chunks.append(t)
                wt[key] = chunks
            else:
                t = wpool.tile(shape, F32, tag=f"wt_{key}")
                nc.sync.dma_start(t[:], ap)
                wt[key] = [t]

        # per-chunk input tiles
        o_rc, d_rc, vd_rc, nrm_rc, t_rc, u_rc = [], [], [], [], [], []
        for h in range(N_CHUNK):
            sl = slice(128 * h, 128 * (h + 1))
            o_t = cpool.tile([128, 3], F32, tag=f"o{h}")
            d_t = cpool.tile([128, 3], F32, tag=f"d{h}")
            n_t = cpool.tile([128, 1], F32, tag=f"n{h}")
            tt_ = cpool.tile([128, NC_], F32, tag=f"t{h}")
            u_t = cpool.tile([128, NF_], F32, tag=f"u{h}")
            nc.sync.dma_start(o_t[:], ins["rays_o"][sl, :])
            nc.sync.dma_start(d_t[:], ins["rays_d"][sl, :])
            nc.sync.dma_start(n_t[:], ins["dnorm"][sl, :])
            nc.sync.dma_start(tt_[:], ins["t_rand"][sl, :])
            nc.sync.dma_start(u_t[:], ins["u_samples"][sl, :])
            o_rc.append(o_t); d_rc.append(d_t); nrm_rc.append(n_t)
            t_rc.append(tt_); u_rc.append(u_t)

        # E_vd [3, 256] all rays (transposed read of viewdirs)
        e_vd = cpool.tile([3, N_RAYS], F32, tag="evd")
        vdap = ins["viewdirs"]
        nc.sync.dma_start(e_vd[:], bass.AP(vdap.tensor, vdap.offset,
                                           [[1, 3], [3, N_RAYS]]))

        zeros256 = cpool.tile([128, NSORT], F32, tag="zeros")
        nc.vector.memset(zeros256[:], 0.0)
        neg_pih = cpool.tile([128, 1], F32, tag="npih")
        nc.vector.memset(neg_pih[:], -PIH)

        # ---------------- ve encode (once): ve [67, 256]
        ve = cpool.tile([67, N_RAYS], F32, tag="ve")
        _encode_trig(nc, pp_enc, epool, wt["enc_ve"][0], e_vd, N_RAYS, ve, neg_pih)
        nc.scalar.copy(ve[64:67, :], e_vd[:])

        # ---------------- z strata per chunk
        z_c, dists_c, mids_c = [], [], []
        for h in range(N_CHUNK):
            z = cpool.tile([128, NC_], F32, tag=f"z{h}")
            nc.vector.tensor_tensor(z[:], t_rc[h][:], wt["width_rep"][0][:], ALU.mult)
            nc.vector.tensor_tensor(z[:], z[:], wt["lower_rep"][0][:], ALU.add)
            dz = cpool.tile([128, NC_], F32, tag=f"dz{h}")
            nc.vector.tensor_tensor(dz[:, 0:NC_ - 1], z[:, 1:NC_], z[:, 0:NC_ - 1], ALU.subtract)
            nc.vector.memset(dz[:, NC_ - 1:NC_], BIG)
            nc.vector.tensor_scalar(dz[:], dz[:], nrm_rc[h][:], None, ALU.mult)
            mid = cpool.tile([128, NC_ - 1], F32, tag=f"mid{h}")
            nc.vector.tensor_tensor(mid[:], z[:, 0:NC_ - 1], z[:, 1:NC_], ALU.add)
            nc.vector.tensor_scalar(mid[:], mid[:], 0.5, None, ALU.mult)
            z_c.append(z); dists_c.append(dz); mids_c.append(mid)

        # coarse pts in ray layout then MLP
        araw_c, rgbraw_c = [], []
        for h in range(N_CHUNK):
            pts = []
            for c in range(3):
                p = cpool.tile([128, NC_], F32, tag=f"cpts{h}{c}")
                nc.vector.tensor_scalar(p[:], z_c[h][:], d_rc[h][:, c:c + 1],
                                        o_rc[h][:, c:c + 1], ALU.mult, ALU.add)
                pts.append(p)
            a_ray = cpool.tile([128, NC_], F32, tag=f"ca{h}")
            rgb_ray = cpool.tile([128, 3 * NC_], F32, tag=f"crgb{h}")
            araw_c.append(a_ray); rgbraw_c.append(rgb_ray)
            for it in range(128 * NC_ // CTILE):     # 16 tiles/chunk, 8 rays each
                r0 = it * (CTILE // NC_)
                _mlp_tile(nc, tc, pools=(epool, apool, spool, pp, pp_enc, pp_sm),
                          wt=wt, net="c", n=CTILE, krep=NC_, rays=8,
                          pts=pts, r0=r0, ray_base=128 * h, ve=ve,
                          a_ray=a_ray, rgb_ray=rgb_ray, neg_pih=neg_pih)

        # coarse composite + hier sampling + sort per chunk, then fine MLP
        out_sb = []
        for h in range(N_CHUNK):
            osb = cpool.tile([128, 8], F32, tag=f"osb{h}")
            out_sb.append(osb)
            w_t = _composite(nc, cpool, spool, h, araw_c[h], rgbraw_c[h],
                             dists_c[h], zeros256, NC_, acc_out=osb[:, 3:4],
                             rgb_out=osb[:, 4:7], tag=f"cw{h}")

            news = _hier_sample(nc, cpool, mpool, spool, h, w_t, mids_c[h],
                                u_rc[h], zeros256, var_out=osb[:, 7:8])

            zf = _sort_zfine(nc, sopool, cpool, h, z_c[h], news)

            # fine pass
            dzf = cpool.tile([128, NT_], F32, tag=f"dzf{h}")
            nc.vector.tensor_tensor(dzf[:, 0:NT_ - 1], zf[:, 1:NT_], zf[:, 0:NT_ - 1], ALU.subtract)
            nc.vector.memset(dzf[:, NT_ - 1:NT_], BIG)
            nc.vector.tensor_scalar(dzf[:], dzf[:], nrm_rc[h][:], None, ALU.mult)

            ptsf = []
            for c in range(3):
                p = cpool.tile([128, NT_], F32, tag=f"fpts{h}{c}")
                nc.vector.tensor_scalar(p[:], zf[:], d_rc[h][:, c:c + 1],
                                        o_rc[h][:, c:c + 1], ALU.mult, ALU.add)
                ptsf.append(p)
            a_rayf = cpool.tile([128, NT_], F32, tag=f"fa{h}")
            rgb_rayf = cpool.tile([128, 3 * NT_], F32, tag=f"frgb{h}")
            for it in range(128 * NT_ // FTILE):     # 64 tiles/chunk, 2 rays each
                r0 = it * (FTILE // NT_)
                _mlp_tile(nc, tc, pools=(epool, apool, spool, pp, pp_enc, pp_sm),
                          wt=wt, net="f", n=FTILE, krep=NT_, rays=2,
                          pts=ptsf, r0=r0, ray_base=128 * h, ve=ve,
                          a_ray=a_rayf, rgb_ray=rgb_rayf, neg_pih=neg_pih)

            _composite(nc, cpool, spool, h, a_rayf, rgb_rayf, dzf, zeros256,
                       NT_, acc_out=None, rgb_out=osb[:, 0:3], tag=f"fw{h}")

            nc.sync.dma_start(out[128 * h:128 * (h + 1), :], osb[:])


def _encode_trig(nc, pp_enc, epool, lhsT, e_tile, n, dest, neg_pih):
    """dest[0:32] = sin(args), dest[32:64] = sin(|r|-pi/2) = -cos(args)."""
    pre = pp_enc.tile([32, n], F32, tag="encps")
    nc.tensor.matmul(pre[:], lhsT[:], e_tile[:], start=True, stop=True)
    k2 = epool.tile([32, n], F32, tag="k2")
    r1 = epool.tile([32, n], F32, tag="r1")
    q = epool.tile([32, n], F32, tag="q")
    nc.vector.tensor_scalar(k2[:], pre[:], INV_2PI, MAGIC, ALU.mult, ALU.add)
    nc.scalar.activation(k2[:], k2[:], AF.Copy, bias=-MAGIC, scale=1.0)
    nc.vector.scalar_tensor_tensor(r1[:], k2[:], -CW1, pre[:], ALU.mult, ALU.add)
    nc.vector.scalar_tensor_tensor(r1[:], k2[:], -CW2, r1[:], ALU.mult, ALU.add)
    nc.vector.tensor_scalar(r1[:], r1[:], PI, -PI, ALU.min, ALU.max)
    nc.scalar.activation(q[:], r1[:], AF.Abs)
    nc.scalar.activation(dest[0:32, :], r1[:], AF.Sin)
    nc.scalar.activation(dest[32:64, :], q[:], AF.Sin, bias=neg_pih[0:32, :])


def _mlp_tile(nc, tc, pools, wt, net, n, krep, rays, pts, r0, ray_base, ve,
              a_ray, rgb_ray, neg_pih):
    epool, apool, spool, pp, pp_enc, pp_sm = pools
    g = lambda k, i=0: wt[f"{net}_{k}"][i]

    # E tile [3, n]
    e = epool.tile([3, n], F32, tag="E")
    for c in range(3):
        nc.sync.dma_start(e[c:c + 1, :], pts[c][r0:r0 + rays, :])

    pe = epool.tile([67, n], F32, tag="pe")
    _encode_trig(nc, pp_enc, epool, wt["enc_pe"][0], e, n, pe, neg_pih)
    nc.scalar.copy(pe[64:67, :], e[:])

    relu_ops = [0]

    def relu(dst, src, bias_ap):
        # alternate ACT / DVE for load balance
        if relu_ops[0] % 2 == 0:
            nc.scalar.activation(dst, src, AF.Relu, bias=bias_ap)
        else:
            nc.vector.tensor_scalar(dst, src, bias_ap, 0.0, ALU.add, ALU.max)
        relu_ops[0] += 1

    def layer(k_chunks, bias_key, m_out, act=True, tag="lay"):
        """k_chunks: list of (lhsT tile, rhs_ap); bias chunks by m."""
        outs = []
        n_m = (m_out + 127) // 128
        for m in range(n_m):
            msz = min(128, m_out - 128 * m)
            ps = pp.tile([msz, n], F32, tag="lp")
            for ki, (lw, rhs) in enumerate(k_chunks):
                nc.tensor.matmul(ps[:], lw[:, 128 * m:128 * m + msz], rhs,
                                 start=(ki == 0), stop=(ki == len(k_chunks) - 1))
            ot = apool.tile([msz, n], F32, tag=f"{tag}{m}")
            bias_ap = g(bias_key, m)[:]
            if act:
                relu(ot[:], ps[:], bias_ap)
            else:
                nc.scalar.activation(ot[:], ps[:], AF.Identity, bias=bias_ap)
            outs.append(ot)
        return outs

    o = layer([(g("w0"), pe[:])], "b0", W, tag="l0")
    for j in range(1, 8):
        if j == 5:
            ch = [(g("w5pe"), pe[:]), (g("w5o", 0), o[0][:]), (g("w5o", 1), o[1][:])]
        else:
            ch = [(g(f"w{j}", 0), o[0][:]), (g(f"w{j}", 1), o[1][:])]
        o = layer(ch, f"b{j}", W, tag=f"l{j % 2}")

    # alpha [1, n]
    ps_a = pp_sm.tile([1, n], F32, tag="psa")
    nc.tensor.matmul(ps_a[:], g("wa", 0), o[0][:], start=True, stop=False)
    nc.tensor.matmul(ps_a[:], g("wa", 1), o[1][:], start=False, stop=True)
    a_st = spool.tile([1, n], F32, tag="ast")
    nc.scalar.activation(a_st[:], ps_a[:], AF.Identity, bias=g("ba_s")[:])

    # feature (no relu)
    feat = layer([(g("wf", 0), o[0][:]), (g("wf", 1), o[1][:])],
                 "bf", W, act=False, tag="ft")

    # view: rhs ve broadcast AP [67, rays*krep]
    vebase = ve[:]
    ve_rhs = bass.AP(vebase.tensor, vebase.offset + ray_base + r0,
                     [list(vebase.ap[0]), [1, rays], [0, krep]])
    h = layer([(g("wvf", 0), feat[0][:]), (g("wvf", 1), feat[1][:]),
               (g("wvv"), ve_rhs)], "bv", 128, tag="vh")[0]

    # rgb [3, n]
    ps_r = pp_sm.tile([3, n], F32, tag="psr")
    nc.tensor.matmul(ps_r[:], g("wr")[:], h[:], start=True, stop=True)
    r_st = spool.tile([3, n], F32, tag="rst")
    nc.scalar.activation(r_st[:], ps_r[:], AF.Identity, bias=g("br")[:])

    # relayout to ray layout
    nc.sync.dma_start(a_ray[r0:r0 + rays, :], a_st[:])
    for c in range(3):
        nc.sync.dma_start(rgb_ray[r0:r0 + rays, krep * c:krep * (c + 1)],
                          r_st[c:c + 1, :])


def _composite(nc, cpool, spool, h, a_ray, rgb_ray, dists, zeros, ns,
               acc_out, rgb_out, tag):
    e = cpool.tile([128, ns], F32, tag=f"{tag}e")
    nc.vector.scalar_tensor_tensor(e[:], a_ray[:], 0.0, dists[:], ALU.max, ALU.mult)
    expne = cpool.tile([128, ns], F32, tag=f"{tag}x")
    nc.scalar.activation(expne[:], e[:], AF.Exp, scale=-1.0)
    alpha = cpool.tile([128, ns], F32, tag=f"{tag}al")
    nc.vector.tensor_scalar(alpha[:], expne[:], -1.0, 1.0, ALU.mult, ALU.add)
    oma = cpool.tile([128, ns], F32, tag=f"{tag}om")
    nc.vector.tensor_scalar(oma[:], alpha[:], -1.0, 1.0, ALU.mult, ALU.add)
    nc.vector.tensor_scalar(oma[:], oma[:], 1e-10, None, ALU.add)
    ti = cpool.tile([128, ns], F32, tag=f"{tag}ti")
    nc.vector.tensor_tensor_scan(ti[:], oma[:], zeros[:, 0:ns], 1.0, ALU.mult, ALU.add)
    w_t = cpool.tile([128, ns], F32, tag=f"{tag}w")
    nc.vector.tensor_copy(w_t[:, 0:1], alpha[:, 0:1])
    nc.vector.tensor_tensor(w_t[:, 1:ns], alpha[:, 1:ns], ti[:, 0:ns - 1], ALU.mult)
    if acc_out is not None:
        nc.vector.tensor_reduce(acc_out, w_t[:], axis=AX.X, op=ALU.add)
    srgb = cpool.tile([128, 3 * ns], F32, tag=f"{tag}sr")
    nc.scalar.activation(srgb[:], rgb_ray[:], AF.Sigmoid)
    sc = spool.tile([128, ns], F32, tag=f"{tag}sc")
    for c in range(3):
        nc.vector.scalar_tensor_tensor(sc[:], srgb[:, ns * c:ns * (c + 1)], 1.0,
                                       w_t[:], ALU.mult, ALU.mult,
                                       accum_out=rgb_out[:, c:c + 1])
    return w_t


def _hier_sample(nc, cpool, mpool, spool, h, w_t, mids, u_t, zeros, var_out):
    NB = NC_ - 1  # 63 bins
    wh = cpool.tile([128, NB - 1], F32, tag=f"wh{h}")
    nc.vector.tensor_scalar(wh[:], w_t[:, 1:NB], 1e-5, None, ALU.add)
    s = cpool.tile([128, 1], F32, tag=f"ws{h}")
    nc.vector.tensor_reduce(s[:], wh[:], axis=AX.X, op=ALU.add)
    rs = cpool.tile([128, 1], F32, tag=f"wr{h}")
    nc.vector.reciprocal(rs[:], s[:])
    wn = cpool.tile([128, NB - 1], F32, tag=f"wn{h}")
    nc.vector.tensor_scalar(wn[:], wh[:], rs[:], None, ALU.mult)
    cdf = cpool.tile([128, NB], F32, tag=f"cdf{h}")
    nc.vector.memset(cdf[:, 0:1], 0.0)
    nc.vector.tensor_tensor_scan(cdf[:, 1:NB], wn[:], zeros[:, 0:NB - 1], 0.0,
                                 ALU.add, ALU.add)

    cl = cpool.tile([128, NF_], F32, tag=f"cl{h}")
    cu = cpool.tile([128, NF_], F32, tag=f"cu{h}")
    ml = cpool.tile([128, NF_], F32, tag=f"ml{h}")
    mu = cpool.tile([128, NF_], F32, tag=f"mu{h}")
    UG = 32
    c3 = cdf[:].rearrange("p (a b) -> p a b", a=1)
    m3 = mids[:].rearrange("p (a b) -> p a b", a=1)
    for gidx in range(NF_ // UG):
        usl = u_t[:, UG * gidx:UG * (gidx + 1)].rearrange("p (a b) -> p a b", b=1)
        ub, cb = bass.broadcast_tensor_aps(usl, c3)
        _, mb = bass.broadcast_tensor_aps(usl, m3)
        mk = mpool.tile([128, UG, NB], F32, tag="mk")
        nc.vector.tensor_tensor(mk[:], ub, cb, ALU.is_ge)
        osl = slice(UG * gidx, UG * (gidx + 1))

        def red(dst, src_b, masked_min):
            sc_ = mpool.tile([128, UG, NB], F32, tag="msrc")
            if masked_min:
                nc.vector.scalar_tensor_tensor(sc_[:], mk[:], BIG, src_b, ALU.mult, ALU.add)
                nc.vector.tensor_reduce(dst.rearrange("p (a b) -> p a b", b=1),
                                        sc_[:], axis=AX.X, op=ALU.min)
            else:
                nc.vector.tensor_tensor(sc_[:], mk[:], src_b, ALU.mult)
                nc.vector.tensor_reduce(dst.rearrange("p (a b) -> p a b", b=1),
                                        sc_[:], axis=AX.X, op=ALU.max)

        red(cl[:, osl], cb, False)
        red(ml[:, osl], mb, False)
        red(cu[:, osl], cb, True)
        red(mu[:, osl], mb, True)

    den = cpool.tile([128, NF_], F32, tag=f"den{h}")
    nc.vector.tensor_tensor(den[:], cu[:], cl[:], ALU.subtract)
    dm = cpool.tile([128, NF_], F32, tag=f"dm{h}")
    nc.vector.tensor_scalar(dm[:], den[:], 1e-5, None, ALU.is_lt)
    nc.vector.tensor_tensor(den[:], den[:], dm[:], ALU.max)
    rden = cpool.tile([128, NF_], F32, tag=f"rd{h}")
    nc.vector.reciprocal(rden[:], den[:])
    t_ = cpool.tile([128, NF_], F32, tag=f"tt{h}")
    nc.vector.tensor_tensor(t_[:], u_t[:], cl[:], ALU.subtract)
    nc.vector.tensor_tensor(t_[:], t_[:], rden[:], ALU.mult)
    dml = cpool.tile([128, NF_], F32, tag=f"dml{h}")
    nc.vector.tensor_tensor(dml[:], mu[:], ml[:], ALU.subtract)
    news = cpool.tile([128, NF_], F32, tag=f"new{h}")
    nc.vector.tensor_tensor(news[:], t_[:], dml[:], ALU.mult)
    nc.vector.tensor_tensor(news[:], news[:], ml[:], ALU.add)

    # z_std -> variance (sqrt on host)
    mean = cpool.tile([128, 1], F32, tag=f"mn{h}")
    nc.vector.tensor_reduce(mean[:], news[:], axis=AX.X, op=ALU.add)
    nc.vector.tensor_scalar(mean[:], mean[:], 1.0 / NF_, None, ALU.mult)
    cen = cpool.tile([128, NF_], F32, tag=f"cn{h}")
    nc.vector.tensor_scalar(cen[:], news[:], mean[:], None, ALU.subtract)
    sq = spool.tile([128, NF_], F32, tag=f"sq{h}")
    ssq = cpool.tile([128, 1], F32, tag=f"ssq{h}")
    nc.vector.scalar_tensor_tensor(sq[:], cen[:], 1.0, cen[:], ALU.mult, ALU.mult,
                                   accum_out=ssq[:])
    nc.vector.tensor_scalar(var_out, ssq[:], 1.0 / NF_, None, ALU.mult)
    return news


def _sort_zfine(nc, sopool, cpool, h, z, news):
    a = sopool.tile([128, NSORT], F32, tag="sa")
    b = sopool.tile([128, NSORT], F32, tag="sb")
    nc.vector.tensor_copy(a[:, 0:NC_], z[:])
    nc.vector.tensor_copy(a[:, NC_:NC_ + NF_], news[:])
    nc.vector.memset(a[:, NT_:NSORT], BIG)
    cur, nxt = a, b
    n = NSORT
    k = 2
    while k <= n:
        j = k // 2
        while j >= 1:
            ko = n // (2 * k) if 2 * k <= n else 1
            bo = k // (2 * j)
            for ksel in (0, 1):
                if 2 * k > n and ksel == 1:
                    break
                asc = (ksel == 0)
                if 2 * k <= n:
                    vw = lambda t, js: t[:].rearrange(
                        "p (a ks b js c) -> p a ks b js c",
                        a=ko, ks=2, b=bo, js=2, c=j)[:, :, ksel, :, js, :]
                else:
                    vw = lambda t, js: t[:].rearrange(
                        "p (b js c) -> p b js c", b=bo, js=2, c=j)[:, :, js, :]
                nc.vector.tensor_tensor(vw(nxt, 0), vw(cur, 0), vw(cur, 1),
                                        ALU.min if asc else ALU.max)
                nc.vector.tensor_tensor(vw(nxt, 1), vw(cur, 0), vw(cur, 1),
                                        ALU.max if asc else ALU.min)
            cur, nxt = nxt, cur
            j //= 2
        k *= 2
    zf = cpool.tile([128, NT_], F32, tag=f"zf{h}")
    nc.vector.tensor_copy(zf[:], cur[:, 0:NT_])
    return zf


# ----------------------------------------------------------------- entry point
_NC_CACHE = {}


def _get_program():
    if "nc" not in _NC_CACHE:
        _NC_CACHE["nc"] = build_program(debug=False)
    return _NC_CACHE["nc"]


def make_in_maps(rays_o, rays_d, t_rand, u_samples, params_coarse, params_fine):
    rays_o = np.asarray(rays_o, np.float32)
    rays_d = np.asarray(rays_d, np.float32)
    t_rand = np.asarray(t_rand, np.float32)
    u_samples = np.asarray(u_samples, np.float32)
    dnorm = np.linalg.norm(rays_d, axis=-1, keepdims=True).astype(np.float32)
    viewdirs = (rays_d / dnorm).astype(np.float32)

    lower, width = _strata_rows()
    lower_rep = np.broadcast_to(lower, (128, NC_)).copy()
    width_rep = np.broadcast_to(width, (128, NC_)).copy()
    enc_pe = _enc_lhsT(L_XYZ, 32)
    enc_ve = _enc_lhsT(L_DIR, 32)

    shared = {"lower_rep": lower_rep, "width_rep": width_rep,
              "enc_pe": enc_pe, "enc_ve": enc_ve}
    for net, params in (("c", params_coarse), ("f", params_fine)):
        t = _host_weights(params)
        ba = t.pop("ba")
        shared[f"{net}_ba_s"] = np.array([[ba]], np.float32)
        for k, v in t.items():
            shared[f"{net}_{k}"] = np.ascontiguousarray(v, np.float32)

    in_maps = []
    for i in range(N_CORES):
        sl = slice(N_RAYS * i, N_RAYS * (i + 1))
        m = dict(shared)
        m["rays_o"] = rays_o[sl]
        m["rays_d"] = rays_d[sl]
        m["viewdirs"] = viewdirs[sl]
        m["dnorm"] = dnorm[sl]
        m["t_rand"] = t_rand[sl]
        m["u_samples"] = u_samples[sl]
        in_maps.append(m)
    return in_maps


def assemble_outputs(results):
    """results: list of per-core dicts with 'out' [256, 8]."""
    outs = np.concatenate([np.asarray(r["out"]) for r in results], axis=0)
    rgb_map = outs[:, 0:3].astype(np.float32)
    acc0 = outs[:, 3].astype(np.float32)
    rgb0 = outs[:, 4:7].astype(np.float32)
    z_std = np.sqrt(outs[:, 7]).astype(np.float32)
    return rgb_map, acc0, rgb0, z_std


def kernel(rays_o, rays_d, t_rand, u_samples, params_coarse, params_fine):
    from concourse.bass_utils import run_bass_kernel_spmd
    nc = _get_program()
    in_maps = make_in_maps(rays_o, rays_d, t_rand, u_samples,
                           params_coarse, params_fine)
    res = run_bass_kernel_spmd(nc, in_maps, core_ids=list(range(N_CORES)))
    return assemble_outputs(res.results)
